# revision 33
# baseline (speedup 1.0000x reference)
"""MoE feed-forward block (B=2, T=2048, D=1024, FF=4096, E=8, top-2) on 8 trn2 cores.

Strategy (expert-parallel, matching the sharding hint):
  - Router (x @ Wr.T, top-2, softmax) computed on host in fp64: it is tiny
    and its output is *indices* + weights, i.e. the dispatch.
  - Dispatch: tokens are gathered per expert on host (the all-to-all), padded
    to a common capacity, and each of the 8 cores runs the FFN of one expert
    over its routed tokens.
  - Combine: host does out[idx_e] += w_e * y_e (fp32), the weighted
    scatter-add, then reshapes to [B, T, D].

Device kernel: both GEMMs run on the PE in fp8 (e4m3) DoubleRow perf mode,
which contracts K=256 per instruction at 0.5 cycles/row -- 4x the fp16 MAC
rate. Plain e4m3 quantization costs ~2.7% error per operand, so (token,
expert) pairs are split into two precision classes by rank of their softmax
combine weight w (per expert, the CP_TARGET highest-w pairs are precise,
the rest cheap -- the error-vs-time frontier of the hi/lo product family is
linear, so a 2-tier split is optimal; the threshold spends the 2e-2 gate
down to ~1.7e-2 measured):

  precise: each GEMM uses a compensated 3-product split
      A @ B ~= A_hi @ B_hi + A_lo @ B_hi + A_hi @ B_lo   (hi/lo both e4m3,
      shared power-of-2 scale, lo = quantized residual; ~7e-4 per operand)
      at 0.75x the fp16 cycle count.
  cheap: single-product pure fp8 at 0.25x the fp16 cycle count. Its ~5%
      FFN error enters the output scaled by w, and only low-w pairs land
      here.

Layouts (pair dim = the DoubleRow K-pair, i.e. k-blocks 2j/2j+1):
  x pairs [j, p, i, cols] with cols = (c0hi | c0lo | c1hi | c1lo | ... |
    cheap-hi): chunk-interleaved so the warmup's chunk-0 hi+lo is ONE
    contiguous DMA slice per j-tile.
  GEMM1 (h = gelu(x @ W1)): psum[f128, ctile] += W1p[j][128,2,f128].T xp[j]
    [128,2,ctile]. Precise: 4 j-tiles x 3 products; ACT gelu -> fp16 h16,
    Pool casts h_hi (e4m3), DVE forms h_lo = h16 - h_hi. Cheap: 4 matmuls,
    ACT gelu straight to e4m3 hq. W1 scaled by 1024 on host; descaled by
    the ACT `scale` operand.
  GEMM2 (y = h @ W2): psum[c128, dtile] += hp[j2][128,2,c128].T W2p[j2]
    [128,2,dtile]; precise 3 products, cheap 1. Each psum is cast to f16
    SBUF (DVE/ACT alternate) and stored by DMAs alternating between the SP
    and ACT queues. Host descales by S_W2 in the combine weights.
  Token-count remainders (CP%128, CQ%128) run transposed (W2 stationary, h
  moving, out [d128, R]) so their matmul cost scales with R, not 128.

Schedule: the serial resources are the per-engine SEQ DMA issue (~1.26us
each), the HWDGE descriptor generator (~0.63us per DMA, global), and the
DMA engines themselves (~332 GB/s aggregate). The cost model's p-state
ramp runs the PE at half clock for its first ~3us and a long idle restarts
it, so ~20 throwaway "pre-warm" matmuls on dependency-free SBUF fill the
window while the first tiles land; real work then starts fully warm at
~4.4us with no PE gap. The warmup covers f-blocks 0..7 of chunk 0,
consuming (x[j] chunk0, W1(q0,j)) stage by stage in DMA arrival order
[j=1,2,3,0]. W1/W2 ride the SP queue in consumption order, x chunk-0
stages ride the ACT queue, x rests follow on SP. G2 stores alternate
between the SP and ACT hwdge queues and the final store is 128 cols so
the tail chain is short.

Verification: CoreSim value-execution (SIM_SAFE=True memsets the pre-warm
operands so the interpreter accepts them; a Gelu shim is needed) matches
the device bit-for-bit on P/Q class errors. CHEAP_IN_WARMUP=True is ~1us
faster but trips a tile-framework dependency race (hq read-before-write,
confirmed by CoreSim) -- keep False.
"""

import sys

sys.path.insert(0, "/opt/trn_rl_repo")

import math
from contextlib import ExitStack

import numpy as np
import ml_dtypes

import concourse.tile as tile
from concourse import bacc, mybir
from concourse.bass_utils import run_bass_kernel_spmd

B, T, D, FF, E, TOPK = 2, 2048, 1024, 4096, 8, 2
N_CORES = 8
FC = FF // 128  # 32 f-blocks
KJ1 = D // 256  # 4 K-pair tiles in GEMM1
KJ2 = FF // 256  # 16 K-pair tiles in GEMM2
S_W1 = 1024.0  # host scale on W1 (power of 2: exact)
S_W2 = 2048.0  # host scale on W2
CP_TARGET = 672  # precise slots per expert (rank-by-weight classification)
N_DUMMY = 17  # PE pre-warm matmuls (N=512 each, ~213/107ns)
SIM_SAFE = False  # memset dummy operands so CoreSim can execute
CHEAP_IN_WARMUP = False  # True trips a tile-framework dependency race (hq read-before-write)

E4NP = ml_dtypes.float8_e4m3

_cache: dict[tuple, object] = {}


def _c_chunks(C: int) -> list[tuple[int, int]]:
    """Split C into <=512-wide moving chunks."""
    out, off = [], 0
    while off < C:
        n = min(512, C - off)
        out.append((off, n))
        off += n
    return out


def _build(CP: int, CQ: int):
    f16 = mybir.dt.float16
    e4 = mybir.dt.float8e4

    nc = bacc.Bacc("TRN2", target_bir_lowering=False, debug=False)
    # x pairs [j, p, i, cols], cols chunk-interleaved (c0hi|c0lo|c1hi|...|Q)
    xhl = nc.dram_tensor("xhl", [KJ1, 128, 2, 2 * CP + CQ], e4, kind="ExternalInput").ap()
    # W1 pairs, f-quarter-major [(q*4+j), p, i, (hi 1024 | lo 1024)],
    # [f'] = 1024*W1[(2j+i)*128+p, q*1024+f']
    w1hl = nc.dram_tensor("w1hl", [4 * KJ1, 128, 2, 2048], e4, kind="ExternalInput").ap()
    # W2 pairs [j2, p, i, (hi 1024 | lo 1024)], [d] = 2048*W2[(2j2+i)*128+p, d]
    w2hl = nc.dram_tensor("w2hl", [KJ2, 128, 2, 2048], e4, kind="ExternalInput").ap()

    outs = {}
    for nm, Cn in (("p", CP), ("q", CQ)):
        CBn, Rn = Cn // 128, Cn % 128
        outs[f"y_{nm}"] = (
            nc.dram_tensor(f"y_{nm}", [CBn * 128, 1024], f16, kind="ExternalOutput").ap()
            if CBn
            else None
        )
        outs[f"yr_{nm}"] = (
            nc.dram_tensor(f"yr_{nm}", [8, 128, Rn], f16, kind="ExternalOutput").ap()
            if Rn
            else None
        )

    with tile.TileContext(nc) as tc:
        _emit(nc, tc, xhl, w1hl, w2hl, outs, CP, CQ)
    nc.compile()
    return nc


def _emit(nc, tc, xhl, w1hl, w2hl, outs, CP, CQ):
    f16 = mybir.dt.float16
    f32 = mybir.dt.float32
    e4 = mybir.dt.float8e4
    GELU = mybir.ActivationFunctionType.Gelu
    DR = mybir.MatmulPerfMode.DoubleRow
    chunksP = _c_chunks(CP)
    chunksQ = _c_chunks(CQ)
    CPB, RP = CP // 128, CP % 128
    CQB, RQ = CQ // 128, CQ % 128
    # chunk holding each class's token remainder (never straddles: chunk
    # edges and CB*128 are both 128-aligned)
    ciRP = (CPB * 128) // 512 if RP else -1
    ciRQ = (CQB * 128) // 512 if RQ else -1
    XW = 2 * CP + CQ
    c0len = chunksP[0][1]
    QB = 2 * CP  # column offset of the cheap class in the x tiles

    with ExitStack() as ctx:
        xp = ctx.enter_context(tc.tile_pool(name="xp", bufs=1))
        w1p = ctx.enter_context(tc.tile_pool(name="w1p", bufs=8))
        w2p = ctx.enter_context(tc.tile_pool(name="w2p", bufs=1))
        hp = ctx.enter_context(tc.tile_pool(name="hp", bufs=1))
        h16p = ctx.enter_context(tc.tile_pool(name="h16p", bufs=8))
        psp = ctx.enter_context(tc.tile_pool(name="psp", bufs=7, space="PSUM"))
        yp = ctx.enter_context(tc.tile_pool(name="yp", bufs=6))

        # --- input DMA. Warmup stages consume (x[j].c0, w1(0,j)) in order
        # j=1,2,3,0: those 8 transfers are issued first (x chunk-0 slices on
        # the ACT queue, W1 on SP). x rests ride DVE; W1 q1..3 + W2 follow
        # on SP in consumption order.
        zwp = ctx.enter_context(tc.tile_pool(name="zwp", bufs=1))
        zw_t = zwp.tile([128, 2, 128], e4, name="zw")
        nc.vector.memset(zw_t[:], 0)
        zero_w = zw_t[:]

        x_t = [xp.tile([128, 2, XW], e4, name=f"x{j}") for j in range(KJ1)]
        jorder = [1, 2, 3, 0]
        w1_t = {}
        for j in jorder:
            nc.scalar.dma_start(x_t[j][:, :, : 2 * c0len], xhl[j][:, :, : 2 * c0len])
        # SP order = consumption order: the 4 warmup W1 tiles (the first
        # one split hi|lo so its hi products can start ~0.7us earlier), then
        # the x rests (needed when the steady loop starts), then the W1/W2
        # stream
        for ji, j in enumerate(jorder):
            t = w1p.tile([128, 2, 2048], e4, tag="w1", name=f"w1_0_{j}")
            if ji == 0:
                nc.sync.dma_start(t[:, :, :1024], w1hl[j][:, :, :1024])
                nc.sync.dma_start(t[:, :, 1024:], w1hl[j][:, :, 1024:])
            else:
                nc.sync.dma_start(t[:], w1hl[j])
            w1_t[0, j] = t
        for j in jorder:
            nc.sync.dma_start(x_t[j][:, :, 2 * c0len :], xhl[j][:, :, 2 * c0len :])
        for q in range(1, 4):
            for j in range(KJ1):
                t = w1p.tile([128, 2, 2048], e4, tag="w1", name=f"w1_{q}_{j}")
                nc.sync.dma_start(t[:], w1hl[q * KJ1 + j])
                w1_t[q, j] = t
        w2_t = []
        for j2 in range(KJ2):
            t = w2p.tile([128, 2, 2048], e4, name=f"w2_{j2}")
            nc.sync.dma_start(t[:], w2hl[j2])
            w2_t.append(t)

        # per-c-chunk h tiles (pair-read dependency boxes stay in-chunk)
        hh_c = [hp.tile([128, FC, cl], e4, name=f"hh{ci}") for ci, (_, cl) in enumerate(chunksP)]
        hl_c = [hp.tile([128, FC, cl], e4, name=f"hl{ci}") for ci, (_, cl) in enumerate(chunksP)]
        hq_c = [hp.tile([128, FC, cl], e4, name=f"hq{ci}") for ci, (_, cl) in enumerate(chunksQ)]

        def w1_slices(fb, j, lo=False):
            t = w1_t[fb // 8, j]
            off = (1024 if lo else 0) + (fb % 8) * 128
            return t[:, :, off : off + 128]

        def g1_precise(ps, fb, coff, clen, j, first, last):
            lh = w1_slices(fb, j)
            ll = w1_slices(fb, j, lo=True)
            rh = x_t[j][:, :, 2 * coff : 2 * coff + clen]
            rl = x_t[j][:, :, 2 * coff + clen : 2 * coff + 2 * clen]
            o = ps[:, :clen]
            nc.tensor.matmul(o, lh, rh, start=first, stop=False, perf_mode=DR)
            nc.tensor.matmul(o, ll, rh, start=False, stop=False, perf_mode=DR)
            nc.tensor.matmul(o, lh, rl, start=False, stop=last, perf_mode=DR)

        def g1_post(ps, fb, ci, clen):
            # one ACT gelu pass (fp16); Pool casts the hi part to e4m3;
            # DVE forms the residual. Spreads the work over three engines.
            h16 = h16p.tile([128, 512], f16, tag="h16", name=f"h16_{fb}_{ci}")
            nc.scalar.activation(h16[:, :clen], ps[:, :clen], GELU, scale=1.0 / S_W1)
            nc.gpsimd.tensor_copy(hh_c[ci][:, fb, :clen], h16[:, :clen])
            nc.vector.tensor_sub(
                hl_c[ci][:, fb, :clen], h16[:, :clen], hh_c[ci][:, fb, :clen]
            )

        def g1_cheap(fb, n_zero=0):
            # j in DMA-arrival order with start=True on the FIRST-arriving
            # product: if the start product's data lands last (ascending j),
            # the accumulation can race with already-ready j-products.
            # n_zero appends zero-contribution products (memset-zero weight):
            # psum-chain-pinned PE delay where an idle gap would otherwise
            # reset the p-state ramp.
            for ci, (coff, clen) in enumerate(chunksQ):
                ps = psp.tile([128, 512], f32, tag="ps", name=f"psq_{fb}_{ci}")
                o = ps[:, :clen]
                for ji, j in enumerate(jorder):
                    rq = x_t[j][:, :, QB + coff : QB + coff + clen]
                    nc.tensor.matmul(
                        o, w1_slices(fb, j), rq,
                        start=(ji == 0), stop=(ji == KJ1 - 1 and n_zero == 0),
                        perf_mode=DR,
                    )
                for z in range(n_zero):
                    nc.tensor.matmul(
                        o, zero_w, rq,
                        start=False, stop=(z == n_zero - 1), perf_mode=DR,
                    )
                nc.scalar.activation(hq_c[ci][:, fb, :clen], o, GELU, scale=1.0 / S_W1)

        # --- PE pre-warm: the p-state model runs the PE at half rate for
        # the first 3us of every continuous-busy run, and ANY idle gap
        # resets the ramp. Burn the pre-data window (~5us: the first x/W1
        # tiles are still in flight) on throwaway matmuls over raw
        # (dependency-free) SBUF so the real work starts fully warm with no
        # gap. Results go to a pool psum tile that is never read.
        if SIM_SAFE:
            # CoreSim refuses reads of uninitialized SBUF; zero the dummy
            # operands (costs ~0.8us of head time, so only for sim debug)
            dmyp = ctx.enter_context(tc.tile_pool(name="dmyp", bufs=1))
            dmy_lt = dmyp.tile([128, 2, 128], e4, name="dmy_l")
            dmy_rt = dmyp.tile([128, 2, 512], e4, name="dmy_r")
            nc.vector.memset(dmy_lt[:], 0)
            nc.vector.memset(dmy_rt[:], 0)
            dummy_lhs = dmy_lt[:]
            dummy_rhs = dmy_rt[:]
        else:
            # raw allocs, no deps: whatever bytes SBUF holds, the products
            # land in a write-only psum bank that nothing ever reads
            dmy_l = nc.alloc_sbuf_tensor("dmy_l", [128, 2, 128], e4)
            dmy_r = nc.alloc_sbuf_tensor("dmy_r", [128, 2, 512], e4)
            dummy_lhs = dmy_l.ap()
            dummy_rhs = dmy_r.ap()
        dmy_p = nc.alloc_psum_tensor("dmy_p", [128, 512], f32)
        ps_dmy = dmy_p.ap()
        # the psum target is raw (non-pool, write-only, never read): the
        # dummies never interact with real psum groups
        for _ in range(N_DUMMY):
            nc.tensor.matmul(ps_dmy[:], dummy_lhs, dummy_rhs,
                             start=True, stop=True, perf_mode=DR)
        for _ in range(2):
            nc.tensor.matmul(ps_dmy[:, :128], dummy_lhs, dummy_rhs[:, :, :128],
                             start=True, stop=True, perf_mode=DR)

        def pe_filler(n):
            # keep the PE busy across a known engine-lag window: an idle PE
            # resets the p-state ramp (3us at half clock), so ~80ns of
            # throwaway work is cheaper than a 40ns gap
            nc.tensor.matmul(ps_dmy[:, :n], dummy_lhs, dummy_rhs[:, :, :n],
                             start=True, stop=True, perf_mode=DR)

        # --- GEMM1. Warmup: j-outer (DMA arrival order) over f-blocks 0..7
        # of chunk 0 -- enough PE work that the x-rest/W2 stream is resident
        # before the steady loop needs it.
        warm_fb = 8
        ps_head = [
            psp.tile([128, 512], f32, tag="ps", name=f"psh_{fb}")
            for fb in range(warm_fb)
        ]
        for ji, j in enumerate(jorder):
            if ji == 0:
                # hi products first: they only need w1(0,j1).hi + x[j1].c0
                for fb in range(warm_fb):
                    nc.tensor.matmul(ps_head[fb][:, :c0len], w1_slices(fb, j),
                                     x_t[j][:, :, :c0len], start=True, stop=False,
                                     perf_mode=DR)
                for fb in range(warm_fb):
                    nc.tensor.matmul(ps_head[fb][:, :c0len], w1_slices(fb, j, lo=True),
                                     x_t[j][:, :, :c0len], start=False, stop=False,
                                     perf_mode=DR)
                    nc.tensor.matmul(ps_head[fb][:, :c0len], w1_slices(fb, j),
                                     x_t[j][:, :, c0len : 2 * c0len], start=False,
                                     stop=False, perf_mode=DR)
            else:
                if ji == 1:
                    # stage j2's tiles land ~0.4us after stage j1's products
                    # drain; idle here would reset the p-state ramp
                    for _ in range(2):
                        nc.tensor.matmul(ps_head[7][:, :c0len], zero_w,
                                         x_t[jorder[0]][:, :, :c0len],
                                         start=False, stop=False, perf_mode=DR)
                for fb in range(warm_fb):
                    g1_precise(ps_head[fb], fb, 0, c0len, j,
                               first=False, last=(ji == KJ1 - 1))
        for fb in range(warm_fb):
            g1_post(ps_head[fb], fb, 0, c0len)
        if CHEAP_IN_WARMUP:
            # the warm f-blocks' cheap groups also run inside the warmup
            # window (their x arrives with the rest DMAs): the steady short
            # groups are then c1-only
            for fb in range(warm_fb):
                g1_cheap(fb)

        # regular groups fb-major (= W1 stream order). Per fb: precise
        # chunks with the remainder-holding chunk first (G2's transposed
        # remainder runs first), then the cheap group.
        idx_chunks = list(enumerate(chunksP))
        if 0 <= ciRP and len(chunksP) > 1:
            idx_chunks = [idx_chunks[ciRP]] + idx_chunks[:ciRP] + idx_chunks[ciRP + 1 :]
        for fb in range(FC):
            for ci, (coff, clen) in idx_chunks:
                if fb < warm_fb and ci == 0:
                    continue
                ps = psp.tile([128, 512], f32, tag="ps", name=f"ps1_{fb}_{ci}")
                for j in range(KJ1):
                    g1_precise(ps, fb, coff, clen, j,
                               first=(j == 0), last=(j == KJ1 - 1))
                g1_post(ps, fb, ci, clen)
            if not (fb < warm_fb and CHEAP_IN_WARMUP):
                g1_cheap(fb)

        # --- GEMM2. Each group's psum is cast to a f16 SBUF tile (DVE and
        # ACT alternate -- both are idle in this phase) and stored by a DMA
        # alternating between the SP and ACT hwdge queues, so consecutive
        # store chains overlap.
        qsel = [0]
        COPY = mybir.ActivationFunctionType.Copy

        def store(dst, src, w):
            k = qsel[0]
            qsel[0] += 1
            ys = yp.tile([128, 512], f16, tag="y", name=f"y{k}")
            if k % 2 == 0:
                nc.vector.tensor_copy(ys[:, :w], src)
            else:
                nc.scalar.activation(ys[:, :w], src, COPY)
            eng = nc.sync if k % 2 == 0 else nc.scalar
            eng.dma_start(dst, ys[:, :w])

        def rem_db(hh_src, hl_src, ci, lo, R, yr, n_prod, db):
            # transposed: W2 stationary, h moving, out [d-block 128, R]
            ps = psp.tile([128, 512], f32, tag="ps", name=f"psr_{db}_{n_prod}")
            o = ps[:, :R]
            for j2 in range(KJ2):
                t2 = w2_t[j2]
                lh = t2[:, :, db * 128 : (db + 1) * 128]
                ll = t2[:, :, 1024 + db * 128 : 1024 + (db + 1) * 128]
                rh = hh_src[ci][:, 2 * j2 : 2 * j2 + 2, lo : lo + R]
                nc.tensor.matmul(o, lh, rh, start=(j2 == 0),
                                 stop=(n_prod == 1 and j2 == KJ2 - 1), perf_mode=DR)
                if n_prod == 3:
                    rl = hl_src[ci][:, 2 * j2 : 2 * j2 + 2, lo : lo + R]
                    nc.tensor.matmul(o, ll, rh, start=False, stop=False, perf_mode=DR)
                    nc.tensor.matmul(o, lh, rl, start=False,
                                     stop=(j2 == KJ2 - 1), perf_mode=DR)
            store(yr[db], o, R)

        def g2_block(hh_src, hl_src, y_dst, chunks_, cb, doff, dlen, n_prod, tag):
            ci = (cb * 128) // 512
            cl = cb * 128 - chunks_[ci][0]
            ps = psp.tile([128, 512], f32, tag="ps", name=f"ps2_{cb}_{doff}_{n_prod}{tag}")
            o = ps[:, :dlen]
            for j2 in range(KJ2):
                t2 = w2_t[j2]
                lh = hh_src[ci][:, 2 * j2 : 2 * j2 + 2, cl : cl + 128]
                rh = t2[:, :, doff : doff + dlen]
                nc.tensor.matmul(o, lh, rh, start=(j2 == 0),
                                 stop=(n_prod == 1 and j2 == KJ2 - 1), perf_mode=DR)
                if n_prod == 3:
                    ll = hl_src[ci][:, 2 * j2 : 2 * j2 + 2, cl : cl + 128]
                    rl = t2[:, :, 1024 + doff : 1024 + doff + dlen]
                    nc.tensor.matmul(o, ll, rh, start=False, stop=False, perf_mode=DR)
                    nc.tensor.matmul(o, lh, rl, start=False,
                                     stop=(j2 == KJ2 - 1), perf_mode=DR)
            store(y_dst[cb * 128 : (cb + 1) * 128, doff : doff + dlen], o, dlen)

        # Emission order: the transposed precise remainder leads (its h deps
        # complete first), then cheap blocks/remainder interleave with the
        # big precise blocks; the final store is 128 cols so the tail chain
        # is short.
        smalls = []
        if RP:
            loP = CPB * 128 - chunksP[ciRP][0]
            smalls += [
                (lambda db=db: rem_db(hh_c, hl_c, ciRP, loP, RP, outs["yr_p"], 3, db))
                for db in range(8)
            ]
        for cqb in range(CQB):
            for doff in (0, 512):
                if CQB and cqb == 0 and doff == 0:
                    continue  # held back for the tail (last_cheap)
                smalls.append(
                    lambda cqb=cqb, doff=doff: g2_block(
                        hq_c, None, outs["y_q"], chunksQ, cqb, doff, 512, 1, ""
                    )
                )
        if RQ:
            loQ = CQB * 128 - chunksQ[ciRQ][0]
            smalls += [
                (lambda db=db: rem_db(hq_c, None, ciRQ, loQ, RQ, outs["yr_q"], 1, db))
                for db in range(8)
            ]

        cbs = sorted(range(CPB), key=lambda cb: (0 if (cb * 128) // 512 == ciRP else 1, cb))
        # hold one cheap block back: it runs between the last big's "a" and
        # "b" sub-blocks so both their store chains hide under PE work and
        # only the tiny "b" store (128 cols) trails the final matmul
        last_cheap = []
        if CQB:
            last_cheap.append(
                lambda: g2_block(hq_c, None, outs["y_q"], chunksQ, 0, 0, 512, 1, "z")
            )
        bigs = []
        for i, cb in enumerate(cbs):
            for doff in (0, 512):
                if i == len(cbs) - 1 and doff == 512:
                    bigs.append(
                        lambda cb=cb: (
                            g2_block(hh_c, hl_c, outs["y_p"], chunksP, cb, 512, 384, 3, "a"),
                            [f() for f in last_cheap],
                            g2_block(hh_c, hl_c, outs["y_p"], chunksP, cb, 896, 128, 3, "b"),
                        )
                    )
                else:
                    bigs.append(
                        lambda cb=cb, doff=doff: g2_block(
                            hh_c, hl_c, outs["y_p"], chunksP, cb, doff, 512, 3, ""
                        )
                    )
        n_lead = min(1, len(bigs) - 1)
        for bg in bigs[:n_lead]:
            bg()
        rest = bigs[n_lead:]
        si = 0
        for k, bg in enumerate(rest):
            # front-load the smalls (3 per big) so the late phase is pure
            # big groups with no small-store chain latencies interspersed
            for _ in range(3):
                if si < len(smalls):
                    smalls[si]()
                    si += 1
            bg()
        while si < len(smalls):
            smalls[si]()
            si += 1


def _route(xf: np.ndarray, Wr: np.ndarray):
    """Host router: top-2 + softmax, fp64 logits for stable decisions."""
    logits = xf.astype(np.float64) @ Wr.astype(np.float64).T  # [N, E]
    top2 = np.argsort(-logits, axis=1, kind="stable")[:, :TOPK]  # [N, 2] desc
    lv = np.take_along_axis(logits, top2, axis=1).astype(np.float32)
    m = lv.max(axis=1, keepdims=True)
    ex = np.exp(lv - m)
    w = (ex / ex.sum(axis=1, keepdims=True)).astype(np.float32)  # [N, 2]
    return top2, w


def _split8(a: np.ndarray, scale: float):
    """hi/lo e4m3 split at a shared (power-of-2) scale."""
    s = (a * scale).astype(np.float32)
    hi = s.astype(E4NP)
    lo = (s - hi.astype(np.float32)).astype(E4NP)
    return hi, lo


def _pack_x(x8h, x8l, idxP, idxQ, CP, CQ):
    """Assemble the [KJ1, 128, 2, 2*CP+CQ] chunk-interleaved x tile."""
    a = np.zeros((2 * CP + CQ, D), dtype=E4NP)
    off = 0
    for coff, clen in _c_chunks(CP):
        sl = idxP[coff : coff + clen]
        a[off : off + len(sl)] = x8h[sl]
        a[off + clen : off + clen + len(sl)] = x8l[sl]
        off += 2 * clen
    a[off : off + len(idxQ)] = x8h[idxQ]
    # d = j*256 + i*128 + p
    C = a.shape[0]
    return np.ascontiguousarray(a.T.reshape(KJ1, 2, 128, C).transpose(0, 2, 1, 3))


def _pack_w1(w: np.ndarray) -> np.ndarray:
    """[1024, 4096] e4m3 -> [4*KJ1, 128, 2, 1024] (f-quarter-major pairs)."""
    a = w.reshape(KJ1, 2, 128, 4, 1024).transpose(3, 0, 2, 1, 4)
    return np.ascontiguousarray(a.reshape(4 * KJ1, 128, 2, 1024))


def _pack_w2(w: np.ndarray) -> np.ndarray:
    """[4096, 1024] e4m3 -> [KJ2, 128, 2, 1024] pair layout."""
    return np.ascontiguousarray(w.reshape(KJ2, 2, 128, 1024).transpose(0, 2, 1, 3))


# SBUF budget: h tiles are 64*CP+32*CQ B/partition + ~120KB fixed.
C_SBUF_MAX = 1200


def _unpack_y(res_e, nm, Cn, ne):
    CBn, Rn = Cn // 128, Cn % 128
    parts = []
    if CBn:
        parts.append(res_e[f"y_{nm}"].astype(np.float32))
    if Rn:
        yre = res_e[f"yr_{nm}"].astype(np.float32)  # [8, 128, Rn]
        parts.append(yre.transpose(2, 0, 1).reshape(Rn, 1024))
    y = parts[0] if len(parts) == 1 else np.concatenate(parts, axis=0)
    return y[:ne]


def _run_pass(x8h, x8l, W1p, W2p, cls, out, trace):
    """One SPMD dispatch over the given per-expert token lists."""
    idxP, wtsP, idxQ, wtsQ = cls
    CP = max(256, (max(len(t) for t in idxP) + 15) // 16 * 16)
    CQ = max(64, (max(len(t) for t in idxQ) + 15) // 16 * 16)

    key = (CP, CQ)
    if key not in _cache:
        _cache[key] = _build(CP, CQ)
    nc = _cache[key]

    in_maps = []
    for e in range(E):
        xhl = _pack_x(x8h, x8l, idxP[e], idxQ[e], CP, CQ)
        in_maps.append({"xhl": xhl, "w1hl": W1p[e], "w2hl": W2p[e]})

    res = run_bass_kernel_spmd(nc, in_maps, list(range(N_CORES)), trace=trace)

    for e in range(E):
        yep = _unpack_y(res.results[e], "p", CP, len(idxP[e]))
        out[idxP[e]] += (wtsP[e] / S_W2)[:, None] * yep
        yeq = _unpack_y(res.results[e], "q", CQ, len(idxQ[e]))
        out[idxQ[e]] += (wtsQ[e] / S_W2)[:, None] * yeq
    return res


def _run(x, Wr, W1, W2, trace=False):
    xf = np.asarray(x, dtype=np.float32).reshape(-1, D)
    N = xf.shape[0]
    top2, tw = _route(xf, np.asarray(Wr, dtype=np.float32))

    # host-side quantization (scales are powers of 2 -> exact descale)
    x8h, x8l = _split8(xf, 1.0)
    W1p, W2p = [], []
    for e in range(E):
        h1, l1 = _split8(np.asarray(W1[e], np.float32), S_W1)
        W1p.append(np.concatenate([_pack_w1(h1), _pack_w1(l1)], axis=3))
        h2, l2 = _split8(np.asarray(W2[e], np.float32), S_W2)
        W2p.append(np.concatenate([_pack_w2(h2), _pack_w2(l2)], axis=3))

    # rank-based 2-tier classification: per expert, the CP_TARGET highest-w
    # pairs are precise, the rest cheap
    idxP, wtsP, idxQ, wtsQ = [], [], [], []
    for e in range(E):
        toks, ws = [], []
        for k in range(TOPK):
            tok = np.nonzero(top2[:, k] == e)[0]
            toks.append(tok)
            ws.append(tw[tok, k])
        tok = np.concatenate(toks)
        w = np.concatenate(ws).astype(np.float32)
        order = np.argsort(-w, kind="stable")
        prec, cheap = order[:CP_TARGET], order[CP_TARGET:]
        idxP.append(tok[prec])
        wtsP.append(w[prec])
        idxQ.append(tok[cheap])
        wtsQ.append(w[cheap])

    cmax = max(max(len(t) for t in idxP), max(len(t) for t in idxQ))
    n_pass = max(1, math.ceil(cmax / C_SBUF_MAX))

    out = np.zeros((N, D), dtype=np.float32)
    res = None
    for p in range(n_pass):
        cls = tuple(
            [t[p * len(t) // n_pass : (p + 1) * len(t) // n_pass] for t in lst]
            for lst in (idxP, wtsP, idxQ, wtsQ)
        )
        res = _run_pass(x8h, x8l, W1p, W2p, cls, out, trace)
    return out.reshape(B, T, D), res


def kernel(x, Wr, W1, W2):
    out, _ = _run(x, Wr, W1, W2, trace=False)
    return out


# revision 34
# speedup vs baseline: 1.0097x; 1.0097x over previous
"""MoE feed-forward block (B=2, T=2048, D=1024, FF=4096, E=8, top-2) on 8 trn2 cores.

Strategy (expert-parallel, matching the sharding hint):
  - Router (x @ Wr.T, top-2, softmax) computed on host in fp64: it is tiny
    and its output is *indices* + weights, i.e. the dispatch.
  - Dispatch: tokens are gathered per expert on host (the all-to-all), padded
    to a common capacity, and each of the 8 cores runs the FFN of one expert
    over its routed tokens.
  - Combine: host does out[idx_e] += w_e * y_e (fp32), the weighted
    scatter-add, then reshapes to [B, T, D].

Device kernel: both GEMMs run on the PE in fp8 (e4m3) DoubleRow perf mode,
which contracts K=256 per instruction at 0.5 cycles/row -- 4x the fp16 MAC
rate. Plain e4m3 quantization costs ~2.7% error per operand, so (token,
expert) pairs are split into two precision classes by rank of their softmax
combine weight w (per expert, the CP_TARGET highest-w pairs are precise,
the rest cheap -- the error-vs-time frontier of the hi/lo product family is
linear, so a 2-tier split is optimal; the threshold spends the 2e-2 gate
down to ~1.7e-2 measured):

  precise: each GEMM uses a compensated 3-product split
      A @ B ~= A_hi @ B_hi + A_lo @ B_hi + A_hi @ B_lo   (hi/lo both e4m3,
      shared power-of-2 scale, lo = quantized residual; ~7e-4 per operand)
      at 0.75x the fp16 cycle count.
  cheap: single-product pure fp8 at 0.25x the fp16 cycle count. Its ~5%
      FFN error enters the output scaled by w, and only low-w pairs land
      here.

Layouts (pair dim = the DoubleRow K-pair, i.e. k-blocks 2j/2j+1):
  x pairs [j, p, i, cols] with cols = (c0hi | c0lo | c1hi | c1lo | ... |
    cheap-hi): chunk-interleaved so the warmup's chunk-0 hi+lo is ONE
    contiguous DMA slice per j-tile.
  GEMM1 (h = gelu(x @ W1)): psum[f128, ctile] += W1p[j][128,2,f128].T xp[j]
    [128,2,ctile]. Precise: 4 j-tiles x 3 products; ACT gelu -> fp16 h16,
    Pool casts h_hi (e4m3), DVE forms h_lo = h16 - h_hi. Cheap: 4 matmuls,
    ACT gelu straight to e4m3 hq. W1 scaled by 1024 on host; descaled by
    the ACT `scale` operand.
  GEMM2 (y = h @ W2): psum[c128, dtile] += hp[j2][128,2,c128].T W2p[j2]
    [128,2,dtile]; precise 3 products, cheap 1. Each psum is cast to f16
    SBUF (DVE/ACT alternate) and stored by DMAs alternating between the SP
    and ACT queues. Host descales by S_W2 in the combine weights.
  Token-count remainders (CP%128, CQ%128) run transposed (W2 stationary, h
  moving, out [d128, R]) so their matmul cost scales with R, not 128.

Schedule: the serial resources are the per-engine SEQ DMA issue (~1.26us
each), the HWDGE descriptor generator (~0.63us per DMA, global), and the
DMA engines themselves (~332 GB/s aggregate). The cost model's p-state
ramp runs the PE at half clock for its first ~3us and a long idle restarts
it, so ~20 throwaway "pre-warm" matmuls on dependency-free SBUF fill the
window while the first tiles land; real work then starts fully warm at
~4.4us with no PE gap. The warmup covers f-blocks 0..7 of chunk 0,
consuming (x[j] chunk0, W1(q0,j)) stage by stage in DMA arrival order
[j=1,2,3,0]. W1/W2 ride the SP queue in consumption order, x chunk-0
stages ride the ACT queue, x rests follow on SP. G2 stores alternate
between the SP and ACT hwdge queues and the final store is 128 cols so
the tail chain is short.

Verification: CoreSim value-execution (SIM_SAFE=True memsets the pre-warm
operands so the interpreter accepts them; a Gelu shim is needed) matches
the device bit-for-bit on P/Q class errors. CHEAP_IN_WARMUP=True is ~1us
faster but trips a tile-framework dependency race (hq read-before-write,
confirmed by CoreSim) -- keep False.
"""

import sys

sys.path.insert(0, "/opt/trn_rl_repo")

import math
from contextlib import ExitStack

import numpy as np
import ml_dtypes

import concourse.tile as tile
from concourse import bacc, mybir
from concourse.bass_utils import run_bass_kernel_spmd

B, T, D, FF, E, TOPK = 2, 2048, 1024, 4096, 8, 2
N_CORES = 8
FC = FF // 128  # 32 f-blocks
KJ1 = D // 256  # 4 K-pair tiles in GEMM1
KJ2 = FF // 256  # 16 K-pair tiles in GEMM2
S_W1 = 1024.0  # host scale on W1 (power of 2: exact)
S_W2 = 2048.0  # host scale on W2
CP_TARGET = 656  # precise slots per expert (rank-by-weight classification)
N_DUMMY = 17  # PE pre-warm matmuls (N=512 each, ~213/107ns)
SIM_SAFE = False  # memset dummy operands so CoreSim can execute
CHEAP_IN_WARMUP = False  # True trips a tile-framework dependency race (hq read-before-write)

E4NP = ml_dtypes.float8_e4m3

_cache: dict[tuple, object] = {}


def _c_chunks(C: int) -> list[tuple[int, int]]:
    """Split C into <=512-wide moving chunks."""
    out, off = [], 0
    while off < C:
        n = min(512, C - off)
        out.append((off, n))
        off += n
    return out


def _build(CP: int, CQ: int):
    f16 = mybir.dt.float16
    e4 = mybir.dt.float8e4

    nc = bacc.Bacc("TRN2", target_bir_lowering=False, debug=False)
    # x pairs [j, p, i, cols], cols chunk-interleaved (c0hi|c0lo|c1hi|...|Q)
    xhl = nc.dram_tensor("xhl", [KJ1, 128, 2, 2 * CP + CQ], e4, kind="ExternalInput").ap()
    # W1 pairs, f-quarter-major [(q*4+j), p, i, (hi 1024 | lo 1024)],
    # [f'] = 1024*W1[(2j+i)*128+p, q*1024+f']
    w1hl = nc.dram_tensor("w1hl", [4 * KJ1, 128, 2, 2048], e4, kind="ExternalInput").ap()
    # W2 pairs [j2, p, i, (hi 1024 | lo 1024)], [d] = 2048*W2[(2j2+i)*128+p, d]
    w2hl = nc.dram_tensor("w2hl", [KJ2, 128, 2, 2048], e4, kind="ExternalInput").ap()

    outs = {}
    for nm, Cn in (("p", CP), ("q", CQ)):
        CBn, Rn = Cn // 128, Cn % 128
        outs[f"y_{nm}"] = (
            nc.dram_tensor(f"y_{nm}", [CBn * 128, 1024], f16, kind="ExternalOutput").ap()
            if CBn
            else None
        )
        outs[f"yr_{nm}"] = (
            nc.dram_tensor(f"yr_{nm}", [8, 128, Rn], f16, kind="ExternalOutput").ap()
            if Rn
            else None
        )

    with tile.TileContext(nc) as tc:
        _emit(nc, tc, xhl, w1hl, w2hl, outs, CP, CQ)
    nc.compile()
    return nc


def _emit(nc, tc, xhl, w1hl, w2hl, outs, CP, CQ):
    f16 = mybir.dt.float16
    f32 = mybir.dt.float32
    e4 = mybir.dt.float8e4
    GELU = mybir.ActivationFunctionType.Gelu
    DR = mybir.MatmulPerfMode.DoubleRow
    chunksP = _c_chunks(CP)
    chunksQ = _c_chunks(CQ)
    CPB, RP = CP // 128, CP % 128
    CQB, RQ = CQ // 128, CQ % 128
    # chunk holding each class's token remainder (never straddles: chunk
    # edges and CB*128 are both 128-aligned)
    ciRP = (CPB * 128) // 512 if RP else -1
    ciRQ = (CQB * 128) // 512 if RQ else -1
    XW = 2 * CP + CQ
    c0len = chunksP[0][1]
    QB = 2 * CP  # column offset of the cheap class in the x tiles

    with ExitStack() as ctx:
        xp = ctx.enter_context(tc.tile_pool(name="xp", bufs=1))
        w1p = ctx.enter_context(tc.tile_pool(name="w1p", bufs=8))
        w2p = ctx.enter_context(tc.tile_pool(name="w2p", bufs=1))
        hp = ctx.enter_context(tc.tile_pool(name="hp", bufs=1))
        h16p = ctx.enter_context(tc.tile_pool(name="h16p", bufs=8))
        psp = ctx.enter_context(tc.tile_pool(name="psp", bufs=7, space="PSUM"))
        yp = ctx.enter_context(tc.tile_pool(name="yp", bufs=6))

        # --- input DMA. Warmup stages consume (x[j].c0, w1(0,j)) in order
        # j=1,2,3,0: those 8 transfers are issued first (x chunk-0 slices on
        # the ACT queue, W1 on SP). x rests ride DVE; W1 q1..3 + W2 follow
        # on SP in consumption order.
        zwp = ctx.enter_context(tc.tile_pool(name="zwp", bufs=1))
        zw_t = zwp.tile([128, 2, 128], e4, name="zw")
        nc.vector.memset(zw_t[:], 0)
        zero_w = zw_t[:]

        x_t = [xp.tile([128, 2, XW], e4, name=f"x{j}") for j in range(KJ1)]
        jorder = [1, 2, 3, 0]
        w1_t = {}
        for j in jorder:
            nc.scalar.dma_start(x_t[j][:, :, : 2 * c0len], xhl[j][:, :, : 2 * c0len])
        # SP order = consumption order: the 4 warmup W1 tiles (the first
        # one split hi|lo so its hi products can start ~0.7us earlier), then
        # the x rests (needed when the steady loop starts), then the W1/W2
        # stream
        for ji, j in enumerate(jorder):
            t = w1p.tile([128, 2, 2048], e4, tag="w1", name=f"w1_0_{j}")
            if ji == 0:
                nc.sync.dma_start(t[:, :, :1024], w1hl[j][:, :, :1024])
                nc.sync.dma_start(t[:, :, 1024:], w1hl[j][:, :, 1024:])
            else:
                nc.sync.dma_start(t[:], w1hl[j])
            w1_t[0, j] = t
        for j in jorder:
            nc.sync.dma_start(x_t[j][:, :, 2 * c0len :], xhl[j][:, :, 2 * c0len :])
        for q in range(1, 4):
            for j in range(KJ1):
                t = w1p.tile([128, 2, 2048], e4, tag="w1", name=f"w1_{q}_{j}")
                nc.sync.dma_start(t[:], w1hl[q * KJ1 + j])
                w1_t[q, j] = t
        w2_t = []
        for j2 in range(KJ2):
            t = w2p.tile([128, 2, 2048], e4, name=f"w2_{j2}")
            nc.sync.dma_start(t[:], w2hl[j2])
            w2_t.append(t)

        # per-c-chunk h tiles (pair-read dependency boxes stay in-chunk)
        hh_c = [hp.tile([128, FC, cl], e4, name=f"hh{ci}") for ci, (_, cl) in enumerate(chunksP)]
        hl_c = [hp.tile([128, FC, cl], e4, name=f"hl{ci}") for ci, (_, cl) in enumerate(chunksP)]
        hq_c = [hp.tile([128, FC, cl], e4, name=f"hq{ci}") for ci, (_, cl) in enumerate(chunksQ)]

        def w1_slices(fb, j, lo=False):
            t = w1_t[fb // 8, j]
            off = (1024 if lo else 0) + (fb % 8) * 128
            return t[:, :, off : off + 128]

        def g1_precise(ps, fb, coff, clen, j, first, last):
            lh = w1_slices(fb, j)
            ll = w1_slices(fb, j, lo=True)
            rh = x_t[j][:, :, 2 * coff : 2 * coff + clen]
            rl = x_t[j][:, :, 2 * coff + clen : 2 * coff + 2 * clen]
            o = ps[:, :clen]
            nc.tensor.matmul(o, lh, rh, start=first, stop=False, perf_mode=DR)
            nc.tensor.matmul(o, ll, rh, start=False, stop=False, perf_mode=DR)
            nc.tensor.matmul(o, lh, rl, start=False, stop=last, perf_mode=DR)

        def g1_post(ps, fb, ci, clen):
            # one ACT gelu pass (fp16); Pool casts the hi part to e4m3;
            # DVE forms the residual. Spreads the work over three engines.
            h16 = h16p.tile([128, 512], f16, tag="h16", name=f"h16_{fb}_{ci}")
            nc.scalar.activation(h16[:, :clen], ps[:, :clen], GELU, scale=1.0 / S_W1)
            nc.gpsimd.tensor_copy(hh_c[ci][:, fb, :clen], h16[:, :clen])
            nc.vector.tensor_sub(
                hl_c[ci][:, fb, :clen], h16[:, :clen], hh_c[ci][:, fb, :clen]
            )

        def g1_cheap(fb, n_zero=0):
            # j in DMA-arrival order with start=True on the FIRST-arriving
            # product: if the start product's data lands last (ascending j),
            # the accumulation can race with already-ready j-products.
            # n_zero appends zero-contribution products (memset-zero weight):
            # psum-chain-pinned PE delay where an idle gap would otherwise
            # reset the p-state ramp.
            for ci, (coff, clen) in enumerate(chunksQ):
                ps = psp.tile([128, 512], f32, tag="ps", name=f"psq_{fb}_{ci}")
                o = ps[:, :clen]
                for ji, j in enumerate(jorder):
                    rq = x_t[j][:, :, QB + coff : QB + coff + clen]
                    nc.tensor.matmul(
                        o, w1_slices(fb, j), rq,
                        start=(ji == 0), stop=(ji == KJ1 - 1 and n_zero == 0),
                        perf_mode=DR,
                    )
                for z in range(n_zero):
                    nc.tensor.matmul(
                        o, zero_w, rq,
                        start=False, stop=(z == n_zero - 1), perf_mode=DR,
                    )
                nc.scalar.activation(hq_c[ci][:, fb, :clen], o, GELU, scale=1.0 / S_W1)

        # --- PE pre-warm: the p-state model runs the PE at half rate for
        # the first 3us of every continuous-busy run, and ANY idle gap
        # resets the ramp. Burn the pre-data window (~5us: the first x/W1
        # tiles are still in flight) on throwaway matmuls over raw
        # (dependency-free) SBUF so the real work starts fully warm with no
        # gap. Results go to a pool psum tile that is never read.
        if SIM_SAFE:
            # CoreSim refuses reads of uninitialized SBUF; zero the dummy
            # operands (costs ~0.8us of head time, so only for sim debug)
            dmyp = ctx.enter_context(tc.tile_pool(name="dmyp", bufs=1))
            dmy_lt = dmyp.tile([128, 2, 128], e4, name="dmy_l")
            dmy_rt = dmyp.tile([128, 2, 512], e4, name="dmy_r")
            nc.vector.memset(dmy_lt[:], 0)
            nc.vector.memset(dmy_rt[:], 0)
            dummy_lhs = dmy_lt[:]
            dummy_rhs = dmy_rt[:]
        else:
            # raw allocs, no deps: whatever bytes SBUF holds, the products
            # land in a write-only psum bank that nothing ever reads
            dmy_l = nc.alloc_sbuf_tensor("dmy_l", [128, 2, 128], e4)
            dmy_r = nc.alloc_sbuf_tensor("dmy_r", [128, 2, 512], e4)
            dummy_lhs = dmy_l.ap()
            dummy_rhs = dmy_r.ap()
        dmy_p = nc.alloc_psum_tensor("dmy_p", [128, 512], f32)
        ps_dmy = dmy_p.ap()
        # the psum target is raw (non-pool, write-only, never read): the
        # dummies never interact with real psum groups
        for _ in range(N_DUMMY):
            nc.tensor.matmul(ps_dmy[:], dummy_lhs, dummy_rhs,
                             start=True, stop=True, perf_mode=DR)
        for _ in range(2):
            nc.tensor.matmul(ps_dmy[:, :128], dummy_lhs, dummy_rhs[:, :, :128],
                             start=True, stop=True, perf_mode=DR)

        def pe_filler(n):
            # keep the PE busy across a known engine-lag window: an idle PE
            # resets the p-state ramp (3us at half clock), so ~80ns of
            # throwaway work is cheaper than a 40ns gap
            nc.tensor.matmul(ps_dmy[:, :n], dummy_lhs, dummy_rhs[:, :, :n],
                             start=True, stop=True, perf_mode=DR)

        # --- GEMM1. Warmup: j-outer (DMA arrival order) over f-blocks 0..7
        # of chunk 0 -- enough PE work that the x-rest/W2 stream is resident
        # before the steady loop needs it.
        warm_fb = 8
        ps_head = [
            psp.tile([128, 512], f32, tag="ps", name=f"psh_{fb}")
            for fb in range(warm_fb)
        ]
        for ji, j in enumerate(jorder):
            if ji == 0:
                # hi products first: they only need w1(0,j1).hi + x[j1].c0
                for fb in range(warm_fb):
                    nc.tensor.matmul(ps_head[fb][:, :c0len], w1_slices(fb, j),
                                     x_t[j][:, :, :c0len], start=True, stop=False,
                                     perf_mode=DR)
                for fb in range(warm_fb):
                    nc.tensor.matmul(ps_head[fb][:, :c0len], w1_slices(fb, j, lo=True),
                                     x_t[j][:, :, :c0len], start=False, stop=False,
                                     perf_mode=DR)
                    nc.tensor.matmul(ps_head[fb][:, :c0len], w1_slices(fb, j),
                                     x_t[j][:, :, c0len : 2 * c0len], start=False,
                                     stop=False, perf_mode=DR)
            else:
                if ji == 1:
                    # stage j2's tiles land ~0.4us after stage j1's products
                    # drain; idle here would reset the p-state ramp
                    for _ in range(2):
                        nc.tensor.matmul(ps_head[7][:, :c0len], zero_w,
                                         x_t[jorder[0]][:, :, :c0len],
                                         start=False, stop=False, perf_mode=DR)
                for fb in range(warm_fb):
                    g1_precise(ps_head[fb], fb, 0, c0len, j,
                               first=False, last=(ji == KJ1 - 1))
        for fb in range(warm_fb):
            g1_post(ps_head[fb], fb, 0, c0len)
        if CHEAP_IN_WARMUP:
            # the warm f-blocks' cheap groups also run inside the warmup
            # window (their x arrives with the rest DMAs): the steady short
            # groups are then c1-only
            for fb in range(warm_fb):
                g1_cheap(fb)

        # regular groups fb-major (= W1 stream order). Per fb: precise
        # chunks with the remainder-holding chunk first (G2's transposed
        # remainder runs first), then the cheap group.
        idx_chunks = list(enumerate(chunksP))
        if 0 <= ciRP and len(chunksP) > 1:
            idx_chunks = [idx_chunks[ciRP]] + idx_chunks[:ciRP] + idx_chunks[ciRP + 1 :]
        for fb in range(FC):
            for ci, (coff, clen) in idx_chunks:
                if fb < warm_fb and ci == 0:
                    continue
                ps = psp.tile([128, 512], f32, tag="ps", name=f"ps1_{fb}_{ci}")
                for j in range(KJ1):
                    g1_precise(ps, fb, coff, clen, j,
                               first=(j == 0), last=(j == KJ1 - 1))
                g1_post(ps, fb, ci, clen)
            if not (fb < warm_fb and CHEAP_IN_WARMUP):
                g1_cheap(fb)

        # --- GEMM2. Each group's psum is cast to a f16 SBUF tile (DVE and
        # ACT alternate -- both are idle in this phase) and stored by a DMA
        # alternating between the SP and ACT hwdge queues, so consecutive
        # store chains overlap.
        qsel = [0]
        COPY = mybir.ActivationFunctionType.Copy

        def store(dst, src, w):
            k = qsel[0]
            qsel[0] += 1
            ys = yp.tile([128, 512], f16, tag="y", name=f"y{k}")
            if k % 2 == 0:
                nc.vector.tensor_copy(ys[:, :w], src)
            else:
                nc.scalar.activation(ys[:, :w], src, COPY)
            eng = nc.sync if k % 2 == 0 else nc.scalar
            eng.dma_start(dst, ys[:, :w])

        def rem_db(hh_src, hl_src, ci, lo, R, yr, n_prod, db):
            # transposed: W2 stationary, h moving, out [d-block 128, R]
            ps = psp.tile([128, 512], f32, tag="ps", name=f"psr_{db}_{n_prod}")
            o = ps[:, :R]
            for j2 in range(KJ2):
                t2 = w2_t[j2]
                lh = t2[:, :, db * 128 : (db + 1) * 128]
                ll = t2[:, :, 1024 + db * 128 : 1024 + (db + 1) * 128]
                rh = hh_src[ci][:, 2 * j2 : 2 * j2 + 2, lo : lo + R]
                nc.tensor.matmul(o, lh, rh, start=(j2 == 0),
                                 stop=(n_prod == 1 and j2 == KJ2 - 1), perf_mode=DR)
                if n_prod == 3:
                    rl = hl_src[ci][:, 2 * j2 : 2 * j2 + 2, lo : lo + R]
                    nc.tensor.matmul(o, ll, rh, start=False, stop=False, perf_mode=DR)
                    nc.tensor.matmul(o, lh, rl, start=False,
                                     stop=(j2 == KJ2 - 1), perf_mode=DR)
            store(yr[db], o, R)

        def g2_block(hh_src, hl_src, y_dst, chunks_, cb, doff, dlen, n_prod, tag):
            ci = (cb * 128) // 512
            cl = cb * 128 - chunks_[ci][0]
            ps = psp.tile([128, 512], f32, tag="ps", name=f"ps2_{cb}_{doff}_{n_prod}{tag}")
            o = ps[:, :dlen]
            for j2 in range(KJ2):
                t2 = w2_t[j2]
                lh = hh_src[ci][:, 2 * j2 : 2 * j2 + 2, cl : cl + 128]
                rh = t2[:, :, doff : doff + dlen]
                nc.tensor.matmul(o, lh, rh, start=(j2 == 0),
                                 stop=(n_prod == 1 and j2 == KJ2 - 1), perf_mode=DR)
                if n_prod == 3:
                    ll = hl_src[ci][:, 2 * j2 : 2 * j2 + 2, cl : cl + 128]
                    rl = t2[:, :, 1024 + doff : 1024 + doff + dlen]
                    nc.tensor.matmul(o, ll, rh, start=False, stop=False, perf_mode=DR)
                    nc.tensor.matmul(o, lh, rl, start=False,
                                     stop=(j2 == KJ2 - 1), perf_mode=DR)
            store(y_dst[cb * 128 : (cb + 1) * 128, doff : doff + dlen], o, dlen)

        # Emission order: the transposed precise remainder leads (its h deps
        # complete first), then cheap blocks/remainder interleave with the
        # big precise blocks; the final store is 128 cols so the tail chain
        # is short.
        smalls = []
        if RP:
            loP = CPB * 128 - chunksP[ciRP][0]
            smalls += [
                (lambda db=db: rem_db(hh_c, hl_c, ciRP, loP, RP, outs["yr_p"], 3, db))
                for db in range(8)
            ]
        for cqb in range(CQB):
            for doff in (0, 512):
                if CQB and cqb == 0 and doff == 0:
                    continue  # held back for the tail (last_cheap)
                smalls.append(
                    lambda cqb=cqb, doff=doff: g2_block(
                        hq_c, None, outs["y_q"], chunksQ, cqb, doff, 512, 1, ""
                    )
                )
        if RQ:
            loQ = CQB * 128 - chunksQ[ciRQ][0]
            smalls += [
                (lambda db=db: rem_db(hq_c, None, ciRQ, loQ, RQ, outs["yr_q"], 1, db))
                for db in range(8)
            ]

        cbs = sorted(range(CPB), key=lambda cb: (0 if (cb * 128) // 512 == ciRP else 1, cb))
        # hold one cheap block back: it runs between the last big's "a" and
        # "b" sub-blocks so both their store chains hide under PE work and
        # only the tiny "b" store (128 cols) trails the final matmul
        last_cheap = []
        if CQB:
            last_cheap.append(
                lambda: g2_block(hq_c, None, outs["y_q"], chunksQ, 0, 0, 512, 1, "z")
            )
        bigs = []
        for i, cb in enumerate(cbs):
            for doff in (0, 512):
                if i == len(cbs) - 1 and doff == 512:
                    bigs.append(
                        lambda cb=cb: (
                            g2_block(hh_c, hl_c, outs["y_p"], chunksP, cb, 512, 384, 3, "a"),
                            [f() for f in last_cheap],
                            g2_block(hh_c, hl_c, outs["y_p"], chunksP, cb, 896, 128, 3, "b"),
                        )
                    )
                else:
                    bigs.append(
                        lambda cb=cb, doff=doff: g2_block(
                            hh_c, hl_c, outs["y_p"], chunksP, cb, doff, 512, 3, ""
                        )
                    )
        n_lead = min(1, len(bigs) - 1)
        for bg in bigs[:n_lead]:
            bg()
        rest = bigs[n_lead:]
        si = 0
        for k, bg in enumerate(rest):
            # front-load the smalls (3 per big) so the late phase is pure
            # big groups with no small-store chain latencies interspersed
            for _ in range(3):
                if si < len(smalls):
                    smalls[si]()
                    si += 1
            bg()
        while si < len(smalls):
            smalls[si]()
            si += 1


def _route(xf: np.ndarray, Wr: np.ndarray):
    """Host router: top-2 + softmax, fp64 logits for stable decisions."""
    logits = xf.astype(np.float64) @ Wr.astype(np.float64).T  # [N, E]
    top2 = np.argsort(-logits, axis=1, kind="stable")[:, :TOPK]  # [N, 2] desc
    lv = np.take_along_axis(logits, top2, axis=1).astype(np.float32)
    m = lv.max(axis=1, keepdims=True)
    ex = np.exp(lv - m)
    w = (ex / ex.sum(axis=1, keepdims=True)).astype(np.float32)  # [N, 2]
    return top2, w


def _split8(a: np.ndarray, scale: float):
    """hi/lo e4m3 split at a shared (power-of-2) scale."""
    s = (a * scale).astype(np.float32)
    hi = s.astype(E4NP)
    lo = (s - hi.astype(np.float32)).astype(E4NP)
    return hi, lo


def _pack_x(x8h, x8l, idxP, idxQ, CP, CQ):
    """Assemble the [KJ1, 128, 2, 2*CP+CQ] chunk-interleaved x tile."""
    a = np.zeros((2 * CP + CQ, D), dtype=E4NP)
    off = 0
    for coff, clen in _c_chunks(CP):
        sl = idxP[coff : coff + clen]
        a[off : off + len(sl)] = x8h[sl]
        a[off + clen : off + clen + len(sl)] = x8l[sl]
        off += 2 * clen
    a[off : off + len(idxQ)] = x8h[idxQ]
    # d = j*256 + i*128 + p
    C = a.shape[0]
    return np.ascontiguousarray(a.T.reshape(KJ1, 2, 128, C).transpose(0, 2, 1, 3))


def _pack_w1(w: np.ndarray) -> np.ndarray:
    """[1024, 4096] e4m3 -> [4*KJ1, 128, 2, 1024] (f-quarter-major pairs)."""
    a = w.reshape(KJ1, 2, 128, 4, 1024).transpose(3, 0, 2, 1, 4)
    return np.ascontiguousarray(a.reshape(4 * KJ1, 128, 2, 1024))


def _pack_w2(w: np.ndarray) -> np.ndarray:
    """[4096, 1024] e4m3 -> [KJ2, 128, 2, 1024] pair layout."""
    return np.ascontiguousarray(w.reshape(KJ2, 2, 128, 1024).transpose(0, 2, 1, 3))


# SBUF budget: h tiles are 64*CP+32*CQ B/partition + ~120KB fixed.
C_SBUF_MAX = 1200


def _unpack_y(res_e, nm, Cn, ne):
    CBn, Rn = Cn // 128, Cn % 128
    parts = []
    if CBn:
        parts.append(res_e[f"y_{nm}"].astype(np.float32))
    if Rn:
        yre = res_e[f"yr_{nm}"].astype(np.float32)  # [8, 128, Rn]
        parts.append(yre.transpose(2, 0, 1).reshape(Rn, 1024))
    y = parts[0] if len(parts) == 1 else np.concatenate(parts, axis=0)
    return y[:ne]


def _run_pass(x8h, x8l, W1p, W2p, cls, out, trace):
    """One SPMD dispatch over the given per-expert token lists."""
    idxP, wtsP, idxQ, wtsQ = cls
    CP = max(256, (max(len(t) for t in idxP) + 15) // 16 * 16)
    CQ = max(64, (max(len(t) for t in idxQ) + 15) // 16 * 16)

    key = (CP, CQ)
    if key not in _cache:
        _cache[key] = _build(CP, CQ)
    nc = _cache[key]

    in_maps = []
    for e in range(E):
        xhl = _pack_x(x8h, x8l, idxP[e], idxQ[e], CP, CQ)
        in_maps.append({"xhl": xhl, "w1hl": W1p[e], "w2hl": W2p[e]})

    res = run_bass_kernel_spmd(nc, in_maps, list(range(N_CORES)), trace=trace)

    for e in range(E):
        yep = _unpack_y(res.results[e], "p", CP, len(idxP[e]))
        out[idxP[e]] += (wtsP[e] / S_W2)[:, None] * yep
        yeq = _unpack_y(res.results[e], "q", CQ, len(idxQ[e]))
        out[idxQ[e]] += (wtsQ[e] / S_W2)[:, None] * yeq
    return res


def _run(x, Wr, W1, W2, trace=False):
    xf = np.asarray(x, dtype=np.float32).reshape(-1, D)
    N = xf.shape[0]
    top2, tw = _route(xf, np.asarray(Wr, dtype=np.float32))

    # host-side quantization (scales are powers of 2 -> exact descale)
    x8h, x8l = _split8(xf, 1.0)
    W1p, W2p = [], []
    for e in range(E):
        h1, l1 = _split8(np.asarray(W1[e], np.float32), S_W1)
        W1p.append(np.concatenate([_pack_w1(h1), _pack_w1(l1)], axis=3))
        h2, l2 = _split8(np.asarray(W2[e], np.float32), S_W2)
        W2p.append(np.concatenate([_pack_w2(h2), _pack_w2(l2)], axis=3))

    # rank-based 2-tier classification: per expert, the CP_TARGET highest-w
    # pairs are precise, the rest cheap
    idxP, wtsP, idxQ, wtsQ = [], [], [], []
    for e in range(E):
        toks, ws = [], []
        for k in range(TOPK):
            tok = np.nonzero(top2[:, k] == e)[0]
            toks.append(tok)
            ws.append(tw[tok, k])
        tok = np.concatenate(toks)
        w = np.concatenate(ws).astype(np.float32)
        order = np.argsort(-w, kind="stable")
        prec, cheap = order[:CP_TARGET], order[CP_TARGET:]
        idxP.append(tok[prec])
        wtsP.append(w[prec])
        idxQ.append(tok[cheap])
        wtsQ.append(w[cheap])

    cmax = max(max(len(t) for t in idxP), max(len(t) for t in idxQ))
    n_pass = max(1, math.ceil(cmax / C_SBUF_MAX))

    out = np.zeros((N, D), dtype=np.float32)
    res = None
    for p in range(n_pass):
        cls = tuple(
            [t[p * len(t) // n_pass : (p + 1) * len(t) // n_pass] for t in lst]
            for lst in (idxP, wtsP, idxQ, wtsQ)
        )
        res = _run_pass(x8h, x8l, W1p, W2p, cls, out, trace)
    return out.reshape(B, T, D), res


def kernel(x, Wr, W1, W2):
    out, _ = _run(x, Wr, W1, W2, trace=False)
    return out
